# revision 2
# baseline (speedup 1.0000x reference)
"""AGNN (3-layer cosine-attention message passing) on 8 trn2 NeuronCores.

Self-contained: host-side graph prep (numpy) + Bass/Tile device program +
run via run_bass_kernel_spmd. kernel(**inputs) takes the full unsharded
inputs and returns the full [G, C] output.

v2: upload-minimized. Per core we upload only its own h shard (f16),
edge src indices (u16), dst-relative slot ids (i8) and graph ids (i8)
(~1.6 MB/core vs ~35 MB in v1). Everything else is built on device:
the replicated node table ([nh*sqrt(beta) | norm' | pad] fp16 rows,
ROW=132) is assembled from the local h shard and AllGathered; the
edge-mask transpose needed for the score matmul is done on the PE with
an identity matmul instead of uploading a precomputed byte mask; the
graph one-hot used for mean-pool readout is built with is_equal against
an iota. h_src is reconstructed per edge as nh*norm' on device, halving
the gather row width. The final [64,128]@[128,100] classifier runs in
numpy on host.
"""

import sys

sys.path.insert(0, "/opt/trn_rl_repo")

import numpy as np

import concourse.bass as bass
import concourse.bacc as bacc
import concourse.mybir as mybir
import concourse.tile as tile

EPS = 1e-12


# ---------------------------------------------------------------- config

class Cfg:
    def __init__(self, N, E, G, NC, blocks_per_sb=3):
        self.N = N                    # real nodes
        self.E = E                    # edges
        self.G = G                    # graphs
        self.NC = NC                  # cores
        self.D = 128
        self.NPC = N // NC            # real nodes per core
        self.BLK = 128
        self.NBLK = -(-self.NPC // self.BLK)       # blocks per core
        self.NPAD = self.NBLK * self.BLK           # padded nodes per core
        self.NPADTOT = self.NPAD * NC
        self.ROW = 132                # [nh*sqrt(beta) 128 | norm' 1 | pad 3]
        self.SBS = blocks_per_sb      # dst blocks per super-block
        self.NSB = -(-self.NBLK // self.SBS)
        self.L = 3


# ---------------------------------------------------------------- host prep

def _prep(cfg, h, src, dst, graph_ids, betas):
    """Build per-core input maps + the shared tile schedule."""
    N, NC, NPC, NPAD, BLK, NBLK = cfg.N, cfg.NC, cfg.NPC, cfg.NPAD, cfg.BLK, cfg.NBLK
    h = np.asarray(h, np.float32)
    src = np.asarray(src, np.int64)
    dst = np.asarray(dst, np.int64)
    graph_ids = np.asarray(graph_ids, np.int64)
    betas = np.asarray(betas, np.float32)

    # edges sorted by global dst -> grouped by (core, local block)
    order = np.argsort(dst, kind="stable")
    e_src = src[order]
    e_dst = dst[order]
    src_pad = (e_src // NPC) * NPAD + (e_src % NPC)
    dcore = e_dst // NPC
    dlocal = e_dst % NPC
    dblk = dlocal // BLK

    # per (core, block) edge counts -> shared tile schedule
    cnt = np.zeros((NC, NBLK), np.int64)
    np.add.at(cnt, (dcore, dblk), 1)
    T_b = np.maximum(1, -(-cnt.max(0) // 128))     # tiles per block (shared)
    Ttot = int(T_b.sum())
    tcol0 = np.zeros(NBLK, np.int64)               # first tile col per block
    tcol0[1:] = np.cumsum(T_b)[:-1]

    sqb = np.zeros((128, 8), np.float32)
    for l in range(cfg.L):
        s = np.sqrt(max(float(betas[l]), 1e-30))
        sqb[:, l] = s
        sqb[:, 4 + l] = 1.0 / s

    in_maps = []
    for c in range(NC):
        hc = h[c * NPC:(c + 1) * NPC]
        rmax = np.maximum(np.abs(hc).max(1, keepdims=True), 1e-20)
        hsh = np.zeros((NPAD, 128), np.int8)
        hsh[:NPC] = np.round(hc * (127.0 / rmax)).astype(np.int8)
        hscale = np.zeros((NPAD, 1), np.float16)
        hscale[:NPC] = (rmax / 127.0).astype(np.float16)

        dummy = c * NPAD + min(NPC, NPAD - 1)      # a zero pad row
        srcidx = np.full((128, Ttot), dummy, np.uint16)
        dstrel = np.full((128, Ttot), -1, np.int8)

        m = dcore == c
        cs, cl, cb = src_pad[m], dlocal[m], dblk[m]
        for b in range(NBLK):
            bm = cb == b
            bs = cs[bm]
            br = (cl[bm] - b * BLK).astype(np.int8)
            n = len(bs)
            t0 = tcol0[b]
            for t in range(T_b[b]):
                lo, hi = t * 128, min((t + 1) * 128, n)
                if lo >= hi:
                    break
                srcidx[0:hi - lo, t0 + t] = bs[lo:hi]
                dstrel[0:hi - lo, t0 + t] = br[lo:hi]

        gid8 = np.full((128, NBLK), -1, np.int8)
        p = np.arange(NPC)
        gid8[p % 128, p // 128] = graph_ids[c * NPC:(c + 1) * NPC]

        in_maps.append(dict(
            hsh=hsh, hscale=hscale, srcidx=srcidx, dstrel=dstrel, gid8=gid8,
            sqb=sqb,
        ))

    counts = np.bincount(graph_ids, minlength=cfg.G).astype(np.float32)
    sched = dict(T_b=[int(x) for x in T_b], tcol0=[int(x) for x in tcol0],
                 Ttot=Ttot)
    return in_maps, counts, sched


# ---------------------------------------------------------------- device program

def build_program(cfg, sched, trace_sim=False):
    f16, f32, i32 = mybir.dt.float16, mybir.dt.float32, mybir.dt.int32
    i8, u16 = mybir.dt.int8, mybir.dt.uint16
    T_b, tcol0 = sched["T_b"], sched["tcol0"]
    Ttot = sched["Ttot"]
    NBLK, SBS, NSB, ROW, G = cfg.NBLK, cfg.SBS, cfg.NSB, cfg.ROW, cfg.G
    Tmax = max(sum(T_b[sb * SBS:(sb + 1) * SBS]) for sb in range(NSB))

    nc = bacc.Bacc("TRN2", target_bir_lowering=False, debug=False,
                   num_devices=cfg.NC)

    hsh_d = nc.dram_tensor("hsh", [cfg.NPAD, 128], i8, kind="ExternalInput").ap()
    hscale_d = nc.dram_tensor("hscale", [cfg.NPAD, 1], f16,
                              kind="ExternalInput").ap()
    srcidx_d = nc.dram_tensor("srcidx", [128, Ttot], u16, kind="ExternalInput").ap()
    dstrel_d = nc.dram_tensor("dstrel", [128, Ttot], i8, kind="ExternalInput").ap()
    gid8_d = nc.dram_tensor("gid8", [128, NBLK], i8, kind="ExternalInput").ap()
    sqb_d = nc.dram_tensor("sqb", [128, 8], f32, kind="ExternalInput").ap()
    pooled_d = nc.dram_tensor("pooled", [G, 128], f32, kind="ExternalOutput").ap()

    pre = nc.dram_tensor("pre", [cfg.NPAD, ROW], f16).ap()
    shard = [nc.dram_tensor(f"shard{l}", [cfg.NPAD, ROW], f16).ap()
             for l in range(cfg.L - 1)]
    localrows = [pre] + shard             # per-layer local (own-node) rows
    tab_space = "Shared" if cfg.NC > 4 else "Local"
    tabs = [nc.dram_tensor(f"tab{l}", [cfg.NPADTOT, ROW], f16,
                           addr_space=tab_space).ap()
            for l in range(cfg.L)]

    groups = [list(range(cfg.NC))]

    from contextlib import ExitStack

    with tile.TileContext(nc, trace_sim=trace_sim) as tc, ExitStack() as ctx:
        const = ctx.enter_context(tc.tile_pool(name="const", bufs=1))
        iota_i = const.tile([128, 128], i32)
        nc.gpsimd.iota(iota_i[:], pattern=[[1, 128]], base=0, channel_multiplier=0)
        iota_f = const.tile([128, 128], f16)
        nc.vector.tensor_copy(iota_f[:], iota_i[:])
        iotac_i = const.tile([128, 1], i32)
        nc.gpsimd.iota(iotac_i[:], pattern=[[0, 1]], base=0, channel_multiplier=1)
        iotac_f = const.tile([128, 1], f16)
        nc.vector.tensor_copy(iotac_f[:], iotac_i[:])

        # identity (f16) for PE transposes: ident[p, j] = (j == p)
        ident = const.tile([128, 128], f16)
        id3 = ident[:].rearrange("p (o j) -> p o j", o=1)
        nc.vector.tensor_tensor(
            out=id3,
            in0=iota_f[:].rearrange("p (o j) -> p o j", o=1),
            in1=iotac_f[:].rearrange("p (o j) -> p o j", j=1)
                .to_broadcast([128, 1, 128]),
            op=mybir.AluOpType.is_equal)

        sqb_s = const.tile([128, 8], f32)
        nc.sync.dma_start(sqb_s[:], sqb_d)

        # edge metadata, converted once and kept resident
        src16 = const.tile([128, Ttot], u16)
        nc.sync.dma_start(src16[:], srcidx_d)
        idx_all = const.tile([128, Ttot], i32)
        nc.vector.tensor_copy(idx_all[:], src16[:])
        drel8 = const.tile([128, Ttot], i8)
        nc.sync.dma_start(drel8[:], dstrel_d)
        drel_all = const.tile([128, Ttot], f16)
        nc.vector.tensor_copy(drel_all[:], drel8[:])

        # graph one-hot for pooling: selg[p, b*G+g] = (gid[p, b] == g)
        gid8_s = const.tile([128, NBLK], i8)
        nc.sync.dma_start(gid8_s[:], gid8_d)
        gid_f = const.tile([128, NBLK], f16)
        nc.vector.tensor_copy(gid_f[:], gid8_s[:])
        selg_s = const.tile([128, NBLK * G], f16)
        nc.vector.tensor_tensor(
            out=selg_s[:].rearrange("p (b g) -> p b g", g=G),
            in0=gid_f[:].rearrange("p (b o) -> p b o", o=1)
                .to_broadcast([128, NBLK, G]),
            in1=iota_f[:, 0:G].rearrange("p (o g) -> p o g", o=1)
                .to_broadcast([128, NBLK, G]),
            op=mybir.AluOpType.is_equal)

        gp = ctx.enter_context(tc.tile_pool(name="gp", bufs=2))
        cp = ctx.enter_context(tc.tile_pool(name="cp", bufs=2))
        ep = ctx.enter_context(tc.tile_pool(name="ep", bufs=2))
        pp = ctx.enter_context(tc.tile_pool(name="pp", bufs=2, space="PSUM"))
        pp2 = ctx.enter_context(tc.tile_pool(name="pp2", bufs=2, space="PSUM"))
        ppool = ctx.enter_context(tc.tile_pool(name="ppool", bufs=1, space="PSUM"))

        pool_ps = ppool.tile([G, 128], f32, tag="pool")

        def store_rows(h3, nrm_eps3, nb, l_next, dram_rows, sb):
            """stg = [h/nrm*sqrt(beta) | nrm/sqrt(beta) | 0 0 0] -> dram."""
            rn = ep.tile([128, SBS], f32, tag="rn")
            rn3 = rn[:, 0:nb].rearrange("p (b o) -> p b o", o=1)
            nc.vector.reciprocal(rn[:, 0:nb], nrm_eps3.rearrange("p b o -> p (b o)"))
            stg = ep.tile([128, SBS * ROW], f16, tag="stg")
            st3 = stg[:, 0:nb * ROW].rearrange("p (b d) -> p b d", d=ROW)
            nc.vector.scalar_tensor_tensor(
                out=st3[:, :, 0:128], in0=h3,
                scalar=sqb_s[:, l_next:l_next + 1],
                in1=rn3.to_broadcast([128, nb, 128]),
                op0=mybir.AluOpType.mult, op1=mybir.AluOpType.mult)
            nc.vector.tensor_tensor(
                out=st3[:, :, 128:129], in0=nrm_eps3,
                in1=sqb_s[:, 4 + l_next:5 + l_next]
                    .rearrange("p (a o) -> p a o", a=1)
                    .to_broadcast([128, nb, 1]),
                op=mybir.AluOpType.mult)
            nc.vector.memset(st3[:, :, 129:132], 0.0)
            out_ap = dram_rows[sb * SBS * 128: sb * SBS * 128 + nb * 128, :] \
                .rearrange("(b p) d -> p b d", p=128)
            nc.sync.dma_start(out_ap, st3)

        # ---- pre-pass: local table rows from the raw h shard
        for sb in range(NSB):
            blocks = list(range(sb * SBS, min((sb + 1) * SBS, NBLK)))
            nb = len(blocks)
            ld8 = ep.tile([128, SBS * 128], i8, tag="ld8")
            l83 = ld8[:, 0:nb * 128].rearrange("p (b d) -> p b d", d=128)
            nc.sync.dma_start(
                l83, hsh_d[sb * SBS * 128: sb * SBS * 128 + nb * 128, :]
                .rearrange("(b p) d -> p b d", p=128))
            rsc = ep.tile([128, SBS], f16, tag="rsc")
            rsc3 = rsc[:, 0:nb].rearrange("p (b o) -> p b o", o=1)
            nc.sync.dma_start(
                rsc3, hscale_d[sb * SBS * 128: sb * SBS * 128 + nb * 128, :]
                .rearrange("(b p) o -> p b o", p=128))
            ldh = ep.tile([128, SBS * 128], f16, tag="ldh")
            ld3 = ldh[:, 0:nb * 128].rearrange("p (b d) -> p b d", d=128)
            nc.vector.tensor_copy(ld3, l83)
            nc.vector.tensor_tensor(
                out=ld3, in0=ld3, in1=rsc3.to_broadcast([128, nb, 128]),
                op=mybir.AluOpType.mult)
            sq = ep.tile([128, SBS * 128], f32, tag="sq")
            q3 = sq[:, 0:nb * 128].rearrange("p (b d) -> p b d", d=128)
            nc.vector.tensor_tensor(out=q3, in0=ld3, in1=ld3,
                                    op=mybir.AluOpType.mult)
            ss = ep.tile([128, SBS], f32, tag="ss")
            nc.vector.tensor_reduce(
                out=ss[:, 0:nb], in_=q3, axis=mybir.AxisListType.X,
                op=mybir.AluOpType.add)
            nrm = ep.tile([128, SBS], f32, tag="nrm")
            nc.scalar.sqrt(nrm[:, 0:nb], ss[:, 0:nb])
            nc.vector.tensor_scalar_add(nrm[:, 0:nb], nrm[:, 0:nb], EPS)
            store_rows(ld3, nrm[:, 0:nb].rearrange("p (b o) -> p b o", o=1),
                       nb, 0, pre, sb)

        nc.gpsimd.collective_compute(
            "AllGather", mybir.AluOpType.bypass, replica_groups=groups,
            ins=[pre[:, :]], outs=[tabs[0][:, :]])

        for l in range(cfg.L):
            tab = tabs[l]
            for sb in range(NSB):
                blocks = list(range(sb * SBS, min((sb + 1) * SBS, NBLK)))
                nb = len(blocks)
                c0 = tcol0[blocks[0]]
                Tsb = sum(T_b[b] for b in blocks)

                tile_bi = []
                for bi, b in enumerate(blocks):
                    tile_bi += [bi] * T_b[b]

                # ---- local dst rows (normalized halves)
                nhblk = ep.tile([128, SBS * 128], f16, tag="nhblk")
                nb3 = nhblk[:, 0:nb * 128].rearrange("p (b d) -> p b d", d=128)
                nc.sync.dma_start(
                    nb3,
                    localrows[l][sb * SBS * 128: sb * SBS * 128 + nb * 128, 0:128]
                    .rearrange("(b p) d -> p b d", p=128))

                # ---- src gather: one [128,1]-offset call per 128-edge tile
                # (HW contract: partition p reads a contiguous line from
                # row idx[p]; multi-column offset APs are NOT honored)
                gsrc = gp.tile([128, Tmax * ROW], f16, tag="gsrc")
                for t in range(Tsb):
                    nc.gpsimd.indirect_dma_start(
                        out=gsrc[:, t * ROW:(t + 1) * ROW], out_offset=None,
                        in_=tab, in_offset=bass.IndirectOffsetOnAxis(
                            ap=idx_all[:, c0 + t:c0 + t + 1], axis=0))
                g3 = gsrc[:, 0:Tsb * ROW].rearrange("p (t d) -> p t d", d=ROW)

                # ---- edge mask:  sel[e, (t, j)] = (j == dstrel[e, t])
                sel = gp.tile([128, Tmax * 128], f16, tag="sel")
                s3 = sel[:, 0:Tsb * 128].rearrange("p (t j) -> p t j", j=128)
                io_b = iota_f[:].rearrange("p (o j) -> p o j", o=1) \
                    .to_broadcast([128, Tsb, 128])
                dr_b = drel_all[:, c0:c0 + Tsb] \
                    .rearrange("p (t o) -> p t o", o=1) \
                    .to_broadcast([128, Tsb, 128])
                nc.vector.tensor_tensor(
                    out=s3, in0=io_b, in1=dr_b, op=mybir.AluOpType.is_equal)

                # ---- reconstructed per-edge value rows [h_src | 1]
                hrhs = gp.tile([128, Tmax * 129], f16, tag="hrhs")
                hr3 = hrhs[:, 0:Tsb * 129].rearrange("p (t d) -> p t d", d=129)
                nc.vector.tensor_tensor(
                    out=hr3[:, :, 0:128], in0=g3[:, :, 0:128],
                    in1=g3[:, :, 128:129].to_broadcast([128, Tsb, 128]),
                    op=mybir.AluOpType.mult)
                nc.vector.memset(hr3[:, :, 128:129], 1.0)

                # ---- scores in groups of 4 tiles (one PSUM bank each):
                # transpose mask on PE, then px[e,:] = nh_dst[dstrel[e],:]
                s_t = cp.tile([128, Tmax], f32, tag="s")
                for g0 in range(0, Tsb, 4):
                    gn = min(4, Tsb - g0)
                    trT = pp2.tile([128, 512], f32, tag="trT")
                    for k in range(gn):
                        nc.tensor.matmul(
                            out=trT[:, k * 128:(k + 1) * 128],
                            lhsT=s3[:, g0 + k, :], rhs=ident[:],
                            start=True, stop=True)
                    selT = cp.tile([128, 512], f16, tag="selT")
                    nc.scalar.copy(out=selT[:, 0:gn * 128],
                                   in_=trT[:, 0:gn * 128])
                    px = pp2.tile([128, 512], f32, tag="px")
                    for k in range(gn):
                        t = g0 + k
                        nc.tensor.matmul(
                            out=px[:, k * 128:(k + 1) * 128],
                            lhsT=selT[:, k * 128:(k + 1) * 128],
                            rhs=nb3[:, tile_bi[t], :],
                            start=True, stop=True)
                    prod = cp.tile([128, 512], f16, tag="prod")
                    p3 = prod[:, 0:gn * 128].rearrange("p (t d) -> p t d", d=128)
                    nc.vector.tensor_tensor(
                        out=p3, in0=px[:, 0:gn * 128].rearrange(
                            "p (t d) -> p t d", d=128),
                        in1=g3[:, g0:g0 + gn, 0:128], op=mybir.AluOpType.mult)
                    nc.vector.tensor_reduce(
                        out=s_t[:, g0:g0 + gn], in_=p3,
                        axis=mybir.AxisListType.X, op=mybir.AluOpType.add)
                a_t = cp.tile([128, Tmax], f16, tag="a")
                nc.scalar.activation(
                    out=a_t[:, 0:Tsb], in_=s_t[:, 0:Tsb],
                    func=mybir.ActivationFunctionType.Exp)

                # ---- masked attention: sel[e, (t, j)] *= a[e, t]
                a_b = a_t[:, 0:Tsb].rearrange("p (t o) -> p t o", o=1) \
                    .to_broadcast([128, Tsb, 128])
                nc.vector.tensor_tensor(
                    out=s3, in0=s3, in1=a_b, op=mybir.AluOpType.mult)

                # ---- scatter:  psum[:, bb*129:(bb+1)*129] += asel_t^T @ [h|1]
                pn = pp.tile([128, 512], f32, tag="pn")
                tt = 0
                for bi, b in enumerate(blocks):
                    for t in range(T_b[b]):
                        nc.tensor.matmul(
                            out=pn[:, bi * 129:bi * 129 + 129],
                            lhsT=s3[:, tt, :],
                            rhs=hr3[:, tt, :],
                            start=(t == 0), stop=(t == T_b[b] - 1))
                        tt += 1

                # ---- epilogue: h' = num / max(den, tiny)
                p3 = pn[:, 0:nb * 129].rearrange("p (b d) -> p b d", d=129)
                den = ep.tile([128, SBS], f32, tag="den")
                nc.vector.tensor_scalar_max(den[:, 0:nb], p3[:, :, 128:129], 1e-30)
                rec = ep.tile([128, SBS], f32, tag="rec")
                nc.vector.reciprocal(rec[:, 0:nb], den[:, 0:nb])
                hsb = ep.tile([128, SBS * 128], f32, tag="hsb")
                h3 = hsb[:, 0:nb * 128].rearrange("p (b d) -> p b d", d=128)
                rec_b = rec[:, 0:nb].rearrange("p (b o) -> p b o", o=1) \
                    .to_broadcast([128, nb, 128])
                nc.vector.tensor_tensor(
                    out=h3, in0=p3[:, :, 0:128], in1=rec_b,
                    op=mybir.AluOpType.mult)

                if l < cfg.L - 1:
                    sq = ep.tile([128, SBS * 128], f32, tag="sq")
                    q3 = sq[:, 0:nb * 128].rearrange("p (b d) -> p b d", d=128)
                    nc.vector.tensor_tensor(out=q3, in0=h3, in1=h3,
                                            op=mybir.AluOpType.mult)
                    ss = ep.tile([128, SBS], f32, tag="ss")
                    nc.vector.tensor_reduce(
                        out=ss[:, 0:nb], in_=q3, axis=mybir.AxisListType.X,
                        op=mybir.AluOpType.add)
                    nrm = ep.tile([128, SBS], f32, tag="nrm")
                    nc.scalar.sqrt(nrm[:, 0:nb], ss[:, 0:nb])
                    nc.vector.tensor_scalar_add(nrm[:, 0:nb], nrm[:, 0:nb], EPS)
                    store_rows(h3, nrm[:, 0:nb].rearrange("p (b o) -> p b o", o=1),
                               nb, l + 1, shard[l], sb)
                else:
                    hf = ep.tile([128, SBS * 128], f16, tag="hf")
                    hf3 = hf[:, 0:nb * 128].rearrange("p (b d) -> p b d", d=128)
                    nc.vector.tensor_copy(out=hf3, in_=h3)
                    for bi, b in enumerate(blocks):
                        nc.tensor.matmul(
                            out=pool_ps[:, :],
                            lhsT=selg_s[:, b * G:b * G + G],
                            rhs=hf3[:, bi, :],
                            start=(b == 0), stop=(b == NBLK - 1))

            if l < cfg.L - 1:
                nc.gpsimd.collective_compute(
                    "AllGather", mybir.AluOpType.bypass,
                    replica_groups=groups,
                    ins=[shard[l][:, :]], outs=[tabs[l + 1][:, :]])

        pooled_s = const.tile([G, 128], f32)
        nc.scalar.copy(out=pooled_s[:, :], in_=pool_ps[:, :])
        nc.sync.dma_start(pooled_d, pooled_s[:, :])

    return nc


# ---------------------------------------------------------------- entry

LAST_EXEC_NS = None
_CACHE = {}


def _get_compiled(cfg, sched):
    key = tuple(sched["T_b"])
    if key not in _CACHE:
        nc = build_program(cfg, sched)
        nc.compile()
        _CACHE[key] = nc
    return _CACHE[key]


def _enable_jax_compile_cache():
    # run_bass_kernel_spmd builds a fresh jit per call; the persistent
    # compilation cache makes the re-compile a disk load (~none of the
    # time is HW) instead of a full XLA compile.
    try:
        import jax
        jax.config.update("jax_compilation_cache_dir", "/tmp/jax_cache")
        jax.config.update("jax_persistent_cache_min_compile_time_secs", 0)
        try:
            jax.config.update("jax_persistent_cache_min_entry_size_bytes", 0)
        except Exception:
            pass
    except Exception:
        pass


def kernel(h, src, dst, graph_ids, betas, W_cls, b_cls, time_execs=0):
    global LAST_EXEC_NS
    import time as _time
    _enable_jax_compile_cache()
    from concourse.bass_utils import run_bass_kernel_spmd

    cfg = Cfg(N=40000, E=640000, G=64, NC=8)
    in_maps, counts, sched = _prep(cfg, h, src, dst, graph_ids, betas)
    nc = _get_compiled(cfg, sched)

    def _run():
        last = None
        for attempt in range(3):
            try:
                return run_bass_kernel_spmd(nc, in_maps,
                                            core_ids=list(range(cfg.NC)))
            except Exception as e:  # transient axon worker hangs
                last = e
                _time.sleep(5)
        raise last

    res = _run()
    if time_execs:
        # no NTFF profiling hook is available in this container, so report
        # median wall-clock of repeated NEFF executions (includes the axon
        # dispatch overhead; on-device time is lower)
        ts = []
        for _ in range(time_execs):
            t0 = _time.time()
            res = run_bass_kernel_spmd(nc, in_maps, core_ids=list(range(cfg.NC)))
            ts.append(_time.time() - t0)
        LAST_EXEC_NS = int(np.median(ts) * 1e9)
    pooled = np.zeros((cfg.G, 128), np.float64)
    for r in res.results:
        pooled[:, :] += r["pooled"][:cfg.G].astype(np.float64)
    hg = (pooled / np.maximum(counts, 1.0)[:, None]).astype(np.float32)
    return hg @ np.asarray(W_cls, np.float32) + np.asarray(b_cls, np.float32)


# revision 3
# speedup vs baseline: 1.4117x; 1.4117x over previous
"""AGNN (3-layer cosine-attention message passing) on 8 trn2 NeuronCores.

Self-contained: host-side graph prep (numpy) + Bass/Tile device program +
run via run_bass_kernel_spmd. kernel(**inputs) takes the full unsharded
inputs and returns the full [G, C] output.

v2: upload-minimized. Per core we upload only its own h shard (f16),
edge src indices (u16), dst-relative slot ids (i8) and graph ids (i8)
(~1.6 MB/core vs ~35 MB in v1). Everything else is built on device:
the replicated node table ([nh*sqrt(beta) | norm' | pad] fp16 rows,
ROW=132) is assembled from the local h shard and AllGathered; the
edge-mask transpose needed for the score matmul is done on the PE with
an identity matmul instead of uploading a precomputed byte mask; the
graph one-hot used for mean-pool readout is built with is_equal against
an iota. h_src is reconstructed per edge as nh*norm' on device, halving
the gather row width. The final [64,128]@[128,100] classifier runs in
numpy on host.
"""

import sys

sys.path.insert(0, "/opt/trn_rl_repo")

import numpy as np

import concourse.bass as bass
import concourse.bacc as bacc
import concourse.mybir as mybir
import concourse.tile as tile

EPS = 1e-12


# ---------------------------------------------------------------- config

class Cfg:
    def __init__(self, N, E, G, NC, blocks_per_sb=3):
        self.N = N                    # real nodes
        self.E = E                    # edges
        self.G = G                    # graphs
        self.NC = NC                  # cores
        self.D = 128
        self.NPC = N // NC            # real nodes per core
        self.BLK = 128
        self.NBLK = -(-self.NPC // self.BLK)       # blocks per core
        self.NPAD = self.NBLK * self.BLK           # padded nodes per core
        self.NPADTOT = self.NPAD * NC
        self.ROW = 132                # [nh*sqrt(beta) 128 | norm' 1 | pad 3]
        self.SBS = blocks_per_sb      # dst blocks per super-block
        self.NSB = -(-self.NBLK // self.SBS)
        self.L = 3


# ---------------------------------------------------------------- host prep

def _prep(cfg, h, src, dst, graph_ids, betas):
    """Build per-core input maps + the shared tile schedule."""
    N, NC, NPC, NPAD, BLK, NBLK = cfg.N, cfg.NC, cfg.NPC, cfg.NPAD, cfg.BLK, cfg.NBLK
    h = np.asarray(h, np.float32)
    src = np.asarray(src, np.int64)
    dst = np.asarray(dst, np.int64)
    graph_ids = np.asarray(graph_ids, np.int64)
    betas = np.asarray(betas, np.float32)

    # edges sorted by global dst -> grouped by (core, local block)
    order = np.argsort(dst, kind="stable")
    e_src = src[order]
    e_dst = dst[order]
    src_pad = (e_src // NPC) * NPAD + (e_src % NPC)
    dcore = e_dst // NPC
    dlocal = e_dst % NPC
    dblk = dlocal // BLK

    # per (core, block) edge counts -> shared tile schedule
    cnt = np.zeros((NC, NBLK), np.int64)
    np.add.at(cnt, (dcore, dblk), 1)
    T_b = np.maximum(1, -(-cnt.max(0) // 128))     # tiles per block (shared)
    Ttot = int(T_b.sum())
    tcol0 = np.zeros(NBLK, np.int64)               # first tile col per block
    tcol0[1:] = np.cumsum(T_b)[:-1]

    sqb = np.zeros((128, 8), np.float32)
    for l in range(cfg.L):
        s = np.sqrt(max(float(betas[l]), 1e-30))
        sqb[:, l] = s
        sqb[:, 4 + l] = 1.0 / s

    in_maps = []
    for c in range(NC):
        hc = h[c * NPC:(c + 1) * NPC]
        rmax = np.maximum(np.abs(hc).max(1, keepdims=True), 1e-20)
        hsh = np.zeros((NPAD, 128), np.int8)
        hsh[:NPC] = np.round(hc * (127.0 / rmax)).astype(np.int8)
        hscale = np.zeros((NPAD, 1), np.float16)
        hscale[:NPC] = (rmax / 127.0).astype(np.float16)

        dummy = c * NPAD + min(NPC, NPAD - 1)      # a zero pad row
        srcidx = np.full((128, Ttot), dummy, np.uint16)
        dstrel = np.full((128, Ttot), -1, np.int8)

        m = dcore == c
        cs, cl, cb = src_pad[m], dlocal[m], dblk[m]
        for b in range(NBLK):
            bm = cb == b
            bs = cs[bm]
            br = (cl[bm] - b * BLK).astype(np.int8)
            n = len(bs)
            t0 = tcol0[b]
            for t in range(T_b[b]):
                lo, hi = t * 128, min((t + 1) * 128, n)
                if lo >= hi:
                    break
                srcidx[0:hi - lo, t0 + t] = bs[lo:hi]
                dstrel[0:hi - lo, t0 + t] = br[lo:hi]

        gid8 = np.full((128, NBLK), -1, np.int8)
        p = np.arange(NPC)
        gid8[p % 128, p // 128] = graph_ids[c * NPC:(c + 1) * NPC]

        in_maps.append(dict(
            hsh=hsh, hscale=hscale, srcidx=srcidx, dstrel=dstrel, gid8=gid8,
            sqb=sqb,
        ))

    counts = np.bincount(graph_ids, minlength=cfg.G).astype(np.float32)
    sched = dict(T_b=[int(x) for x in T_b], tcol0=[int(x) for x in tcol0],
                 Ttot=Ttot)
    return in_maps, counts, sched


# ---------------------------------------------------------------- device program

def build_program(cfg, sched, trace_sim=False):
    f16, f32, i32 = mybir.dt.float16, mybir.dt.float32, mybir.dt.int32
    i8, u16 = mybir.dt.int8, mybir.dt.uint16
    T_b, tcol0 = sched["T_b"], sched["tcol0"]
    Ttot = sched["Ttot"]
    NBLK, SBS, NSB, ROW, G = cfg.NBLK, cfg.SBS, cfg.NSB, cfg.ROW, cfg.G
    Tmax = max(sum(T_b[sb * SBS:(sb + 1) * SBS]) for sb in range(NSB))

    nc = bacc.Bacc("TRN2", target_bir_lowering=False, debug=False,
                   num_devices=cfg.NC)

    hsh_d = nc.dram_tensor("hsh", [cfg.NPAD, 128], i8, kind="ExternalInput").ap()
    hscale_d = nc.dram_tensor("hscale", [cfg.NPAD, 1], f16,
                              kind="ExternalInput").ap()
    srcidx_d = nc.dram_tensor("srcidx", [128, Ttot], u16, kind="ExternalInput").ap()
    dstrel_d = nc.dram_tensor("dstrel", [128, Ttot], i8, kind="ExternalInput").ap()
    gid8_d = nc.dram_tensor("gid8", [128, NBLK], i8, kind="ExternalInput").ap()
    sqb_d = nc.dram_tensor("sqb", [128, 8], f32, kind="ExternalInput").ap()
    pooled_d = nc.dram_tensor("pooled", [G, 128], f32, kind="ExternalOutput").ap()

    pre = nc.dram_tensor("pre", [cfg.NPAD, ROW], f16).ap()
    shard = [nc.dram_tensor(f"shard{l}", [cfg.NPAD, ROW], f16).ap()
             for l in range(cfg.L - 1)]
    localrows = [pre] + shard             # per-layer local (own-node) rows
    tab_space = "Shared" if cfg.NC > 4 else "Local"
    tabs = [nc.dram_tensor(f"tab{l}", [cfg.NPADTOT, ROW], f16,
                           addr_space=tab_space).ap()
            for l in range(cfg.L)]

    groups = [list(range(cfg.NC))]

    from contextlib import ExitStack

    with tile.TileContext(nc, trace_sim=trace_sim) as tc, ExitStack() as ctx:
        const = ctx.enter_context(tc.tile_pool(name="const", bufs=1))
        iota_i = const.tile([128, 128], i32)
        nc.gpsimd.iota(iota_i[:], pattern=[[1, 128]], base=0, channel_multiplier=0)
        iota_f = const.tile([128, 128], f16)
        nc.vector.tensor_copy(iota_f[:], iota_i[:])
        iotac_i = const.tile([128, 1], i32)
        nc.gpsimd.iota(iotac_i[:], pattern=[[0, 1]], base=0, channel_multiplier=1)
        iotac_f = const.tile([128, 1], f16)
        nc.vector.tensor_copy(iotac_f[:], iotac_i[:])

        # identity (f16) for PE transposes: ident[p, j] = (j == p)
        ident = const.tile([128, 128], f16)
        id3 = ident[:].rearrange("p (o j) -> p o j", o=1)
        nc.vector.tensor_tensor(
            out=id3,
            in0=iota_f[:].rearrange("p (o j) -> p o j", o=1),
            in1=iotac_f[:].rearrange("p (o j) -> p o j", j=1)
                .to_broadcast([128, 1, 128]),
            op=mybir.AluOpType.is_equal)

        sqb_s = const.tile([128, 8], f32)
        nc.sync.dma_start(sqb_s[:], sqb_d)

        # edge metadata, converted once and kept resident
        src16 = const.tile([128, Ttot], u16)
        nc.sync.dma_start(src16[:], srcidx_d)
        idx_all = const.tile([128, Ttot], i32)
        nc.vector.tensor_copy(idx_all[:], src16[:])
        drel8 = const.tile([128, Ttot], i8)
        nc.sync.dma_start(drel8[:], dstrel_d)
        drel_all = const.tile([128, Ttot], f16)
        nc.vector.tensor_copy(drel_all[:], drel8[:])

        # graph one-hot for pooling: selg[p, b*G+g] = (gid[p, b] == g)
        gid8_s = const.tile([128, NBLK], i8)
        nc.sync.dma_start(gid8_s[:], gid8_d)
        gid_f = const.tile([128, NBLK], f16)
        nc.vector.tensor_copy(gid_f[:], gid8_s[:])
        selg_s = const.tile([128, NBLK * G], f16)
        nc.vector.tensor_tensor(
            out=selg_s[:].rearrange("p (b g) -> p b g", g=G),
            in0=gid_f[:].rearrange("p (b o) -> p b o", o=1)
                .to_broadcast([128, NBLK, G]),
            in1=iota_f[:, 0:G].rearrange("p (o g) -> p o g", o=1)
                .to_broadcast([128, NBLK, G]),
            op=mybir.AluOpType.is_equal)

        gp = ctx.enter_context(tc.tile_pool(name="gp", bufs=2))
        cp = ctx.enter_context(tc.tile_pool(name="cp", bufs=2))
        ep = ctx.enter_context(tc.tile_pool(name="ep", bufs=2))
        pp = ctx.enter_context(tc.tile_pool(name="pp", bufs=2, space="PSUM"))
        pp2 = ctx.enter_context(tc.tile_pool(name="pp2", bufs=2, space="PSUM"))
        ppool = ctx.enter_context(tc.tile_pool(name="ppool", bufs=1, space="PSUM"))

        pool_ps = ppool.tile([G, 128], f32, tag="pool")

        def store_rows(h3, nrm_eps3, nb, l_next, dram_rows, sb):
            """stg = [h/nrm*sqrt(beta) | nrm/sqrt(beta) | 0 0 0] -> dram."""
            rn = ep.tile([128, SBS], f32, tag="rn")
            rn3 = rn[:, 0:nb].rearrange("p (b o) -> p b o", o=1)
            nc.vector.reciprocal(rn[:, 0:nb], nrm_eps3.rearrange("p b o -> p (b o)"))
            stg = ep.tile([128, SBS * ROW], f16, tag="stg")
            st3 = stg[:, 0:nb * ROW].rearrange("p (b d) -> p b d", d=ROW)
            nc.vector.scalar_tensor_tensor(
                out=st3[:, :, 0:128], in0=h3,
                scalar=sqb_s[:, l_next:l_next + 1],
                in1=rn3.to_broadcast([128, nb, 128]),
                op0=mybir.AluOpType.mult, op1=mybir.AluOpType.mult)
            nc.vector.tensor_tensor(
                out=st3[:, :, 128:129], in0=nrm_eps3,
                in1=sqb_s[:, 4 + l_next:5 + l_next]
                    .rearrange("p (a o) -> p a o", a=1)
                    .to_broadcast([128, nb, 1]),
                op=mybir.AluOpType.mult)
            nc.vector.memset(st3[:, :, 129:132], 0.0)
            out_ap = dram_rows[sb * SBS * 128: sb * SBS * 128 + nb * 128, :] \
                .rearrange("(b p) d -> p b d", p=128)
            nc.sync.dma_start(out_ap, st3)

        # ---- pre-pass: local table rows from the raw h shard
        for sb in range(NSB):
            blocks = list(range(sb * SBS, min((sb + 1) * SBS, NBLK)))
            nb = len(blocks)
            ld8 = ep.tile([128, SBS * 128], i8, tag="ld8")
            l83 = ld8[:, 0:nb * 128].rearrange("p (b d) -> p b d", d=128)
            nc.sync.dma_start(
                l83, hsh_d[sb * SBS * 128: sb * SBS * 128 + nb * 128, :]
                .rearrange("(b p) d -> p b d", p=128))
            rsc = ep.tile([128, SBS], f16, tag="rsc")
            rsc3 = rsc[:, 0:nb].rearrange("p (b o) -> p b o", o=1)
            nc.sync.dma_start(
                rsc3, hscale_d[sb * SBS * 128: sb * SBS * 128 + nb * 128, :]
                .rearrange("(b p) o -> p b o", p=128))
            ldh = ep.tile([128, SBS * 128], f16, tag="ldh")
            ld3 = ldh[:, 0:nb * 128].rearrange("p (b d) -> p b d", d=128)
            nc.vector.tensor_copy(ld3, l83)
            nc.vector.tensor_tensor(
                out=ld3, in0=ld3, in1=rsc3.to_broadcast([128, nb, 128]),
                op=mybir.AluOpType.mult)
            sq = ep.tile([128, SBS * 128], f32, tag="sq")
            q3 = sq[:, 0:nb * 128].rearrange("p (b d) -> p b d", d=128)
            nc.vector.tensor_tensor(out=q3, in0=ld3, in1=ld3,
                                    op=mybir.AluOpType.mult)
            ss = ep.tile([128, SBS], f32, tag="ss")
            nc.vector.tensor_reduce(
                out=ss[:, 0:nb], in_=q3, axis=mybir.AxisListType.X,
                op=mybir.AluOpType.add)
            nrm = ep.tile([128, SBS], f32, tag="nrm")
            nc.scalar.sqrt(nrm[:, 0:nb], ss[:, 0:nb])
            nc.vector.tensor_scalar_add(nrm[:, 0:nb], nrm[:, 0:nb], EPS)
            store_rows(ld3, nrm[:, 0:nb].rearrange("p (b o) -> p b o", o=1),
                       nb, 0, pre, sb)

        nc.gpsimd.collective_compute(
            "AllGather", mybir.AluOpType.bypass, replica_groups=groups,
            ins=[pre[:, :]], outs=[tabs[0][:, :]])

        for l in range(cfg.L):
            tab = tabs[l]
            for sb in range(NSB):
                blocks = list(range(sb * SBS, min((sb + 1) * SBS, NBLK)))
                nb = len(blocks)
                c0 = tcol0[blocks[0]]
                Tsb = sum(T_b[b] for b in blocks)

                tile_bi = []
                for bi, b in enumerate(blocks):
                    tile_bi += [bi] * T_b[b]

                # ---- local dst rows (normalized halves)
                nhblk = ep.tile([128, SBS * 128], f16, tag="nhblk")
                nb3 = nhblk[:, 0:nb * 128].rearrange("p (b d) -> p b d", d=128)
                nc.sync.dma_start(
                    nb3,
                    localrows[l][sb * SBS * 128: sb * SBS * 128 + nb * 128, 0:128]
                    .rearrange("(b p) d -> p b d", p=128))

                # ---- src gather: one [128,1]-offset call per 128-edge tile
                # (HW contract: partition p reads a contiguous line from
                # row idx[p]; multi-column offset APs are NOT honored)
                gsrc = gp.tile([128, Tmax * ROW], f16, tag="gsrc")
                for t in range(Tsb):
                    nc.gpsimd.indirect_dma_start(
                        out=gsrc[:, t * ROW:(t + 1) * ROW], out_offset=None,
                        in_=tab, in_offset=bass.IndirectOffsetOnAxis(
                            ap=idx_all[:, c0 + t:c0 + t + 1], axis=0))
                g3 = gsrc[:, 0:Tsb * ROW].rearrange("p (t d) -> p t d", d=ROW)

                # ---- edge mask:  sel[e, (t, j)] = (j == dstrel[e, t])
                sel = gp.tile([128, Tmax * 128], f16, tag="sel")
                s3 = sel[:, 0:Tsb * 128].rearrange("p (t j) -> p t j", j=128)
                io_b = iota_f[:].rearrange("p (o j) -> p o j", o=1) \
                    .to_broadcast([128, Tsb, 128])
                dr_b = drel_all[:, c0:c0 + Tsb] \
                    .rearrange("p (t o) -> p t o", o=1) \
                    .to_broadcast([128, Tsb, 128])
                nc.vector.tensor_tensor(
                    out=s3, in0=io_b, in1=dr_b, op=mybir.AluOpType.is_equal)

                # ---- reconstructed per-edge value rows [h_src | 1]
                hrhs = gp.tile([128, Tmax * 129], f16, tag="hrhs")
                hr3 = hrhs[:, 0:Tsb * 129].rearrange("p (t d) -> p t d", d=129)
                nc.vector.tensor_tensor(
                    out=hr3[:, :, 0:128], in0=g3[:, :, 0:128],
                    in1=g3[:, :, 128:129].to_broadcast([128, Tsb, 128]),
                    op=mybir.AluOpType.mult)
                nc.vector.memset(hr3[:, :, 128:129], 1.0)

                # ---- scores in groups of 4 tiles (one PSUM bank each):
                # transpose mask on PE, then px[e,:] = nh_dst[dstrel[e],:]
                s_t = cp.tile([128, Tmax], f32, tag="s")
                for g0 in range(0, Tsb, 4):
                    gn = min(4, Tsb - g0)
                    trT = pp2.tile([128, 512], f32, tag="trT")
                    for k in range(gn):
                        nc.tensor.matmul(
                            out=trT[:, k * 128:(k + 1) * 128],
                            lhsT=s3[:, g0 + k, :], rhs=ident[:],
                            start=True, stop=True)
                    selT = cp.tile([128, 512], f16, tag="selT")
                    nc.scalar.copy(out=selT[:, 0:gn * 128],
                                   in_=trT[:, 0:gn * 128])
                    px = pp2.tile([128, 512], f32, tag="px")
                    for k in range(gn):
                        t = g0 + k
                        nc.tensor.matmul(
                            out=px[:, k * 128:(k + 1) * 128],
                            lhsT=selT[:, k * 128:(k + 1) * 128],
                            rhs=nb3[:, tile_bi[t], :],
                            start=True, stop=True)
                    prod = cp.tile([128, 512], f16, tag="prod")
                    p3 = prod[:, 0:gn * 128].rearrange("p (t d) -> p t d", d=128)
                    nc.vector.tensor_tensor(
                        out=p3, in0=px[:, 0:gn * 128].rearrange(
                            "p (t d) -> p t d", d=128),
                        in1=g3[:, g0:g0 + gn, 0:128], op=mybir.AluOpType.mult)
                    nc.vector.tensor_reduce(
                        out=s_t[:, g0:g0 + gn], in_=p3,
                        axis=mybir.AxisListType.X, op=mybir.AluOpType.add)
                a_t = cp.tile([128, Tmax], f16, tag="a")
                nc.scalar.activation(
                    out=a_t[:, 0:Tsb], in_=s_t[:, 0:Tsb],
                    func=mybir.ActivationFunctionType.Exp)

                # ---- masked attention: sel[e, (t, j)] *= a[e, t]
                a_b = a_t[:, 0:Tsb].rearrange("p (t o) -> p t o", o=1) \
                    .to_broadcast([128, Tsb, 128])
                nc.vector.tensor_tensor(
                    out=s3, in0=s3, in1=a_b, op=mybir.AluOpType.mult)

                # ---- scatter:  psum[:, bb*129:(bb+1)*129] += asel_t^T @ [h|1]
                pn = pp.tile([128, 512], f32, tag="pn")
                tt = 0
                for bi, b in enumerate(blocks):
                    for t in range(T_b[b]):
                        nc.tensor.matmul(
                            out=pn[:, bi * 129:bi * 129 + 129],
                            lhsT=s3[:, tt, :],
                            rhs=hr3[:, tt, :],
                            start=(t == 0), stop=(t == T_b[b] - 1))
                        tt += 1

                # ---- epilogue: h' = num / max(den, tiny)
                p3 = pn[:, 0:nb * 129].rearrange("p (b d) -> p b d", d=129)
                den = ep.tile([128, SBS], f32, tag="den")
                nc.vector.tensor_scalar_max(den[:, 0:nb], p3[:, :, 128:129], 1e-30)
                rec = ep.tile([128, SBS], f32, tag="rec")
                nc.vector.reciprocal(rec[:, 0:nb], den[:, 0:nb])
                hsb = ep.tile([128, SBS * 128], f32, tag="hsb")
                h3 = hsb[:, 0:nb * 128].rearrange("p (b d) -> p b d", d=128)
                rec_b = rec[:, 0:nb].rearrange("p (b o) -> p b o", o=1) \
                    .to_broadcast([128, nb, 128])
                nc.vector.tensor_tensor(
                    out=h3, in0=p3[:, :, 0:128], in1=rec_b,
                    op=mybir.AluOpType.mult)

                if l < cfg.L - 1:
                    sq = ep.tile([128, SBS * 128], f32, tag="sq")
                    q3 = sq[:, 0:nb * 128].rearrange("p (b d) -> p b d", d=128)
                    nc.vector.tensor_tensor(out=q3, in0=h3, in1=h3,
                                            op=mybir.AluOpType.mult)
                    ss = ep.tile([128, SBS], f32, tag="ss")
                    nc.vector.tensor_reduce(
                        out=ss[:, 0:nb], in_=q3, axis=mybir.AxisListType.X,
                        op=mybir.AluOpType.add)
                    nrm = ep.tile([128, SBS], f32, tag="nrm")
                    nc.scalar.sqrt(nrm[:, 0:nb], ss[:, 0:nb])
                    nc.vector.tensor_scalar_add(nrm[:, 0:nb], nrm[:, 0:nb], EPS)
                    store_rows(h3, nrm[:, 0:nb].rearrange("p (b o) -> p b o", o=1),
                               nb, l + 1, shard[l], sb)
                else:
                    hf = ep.tile([128, SBS * 128], f16, tag="hf")
                    hf3 = hf[:, 0:nb * 128].rearrange("p (b d) -> p b d", d=128)
                    nc.vector.tensor_copy(out=hf3, in_=h3)
                    for bi, b in enumerate(blocks):
                        nc.tensor.matmul(
                            out=pool_ps[:, :],
                            lhsT=selg_s[:, b * G:b * G + G],
                            rhs=hf3[:, bi, :],
                            start=(b == 0), stop=(b == NBLK - 1))

            if l < cfg.L - 1:
                nc.gpsimd.collective_compute(
                    "AllGather", mybir.AluOpType.bypass,
                    replica_groups=groups,
                    ins=[shard[l][:, :]], outs=[tabs[l + 1][:, :]])

        pooled_s = const.tile([G, 128], f32)
        nc.scalar.copy(out=pooled_s[:, :], in_=pool_ps[:, :])
        nc.sync.dma_start(pooled_d, pooled_s[:, :])

    return nc


# ---------------------------------------------------------------- entry

LAST_EXEC_NS = None
_CACHE = {}


def _get_compiled(cfg, sched):
    key = tuple(sched["T_b"])
    if key not in _CACHE:
        nc = build_program(cfg, sched)
        nc.compile()
        # the program is frozen after compile; memoize the BIR serialization
        # that the bass_exec lowering re-runs on every dispatch
        raw = nc.to_json_bytes()
        nc.to_json_bytes = lambda: raw
        _CACHE[key] = nc
    return _CACHE[key]


def _enable_jax_compile_cache():
    # run_bass_kernel_spmd builds a fresh jit per call; the persistent
    # compilation cache makes the re-compile a disk load (~none of the
    # time is HW) instead of a full XLA compile.
    try:
        import jax
        jax.config.update("jax_compilation_cache_dir", "/tmp/jax_cache")
        jax.config.update("jax_persistent_cache_min_compile_time_secs", 0)
        try:
            jax.config.update("jax_persistent_cache_min_entry_size_bytes", 0)
        except Exception:
            pass
    except Exception:
        pass


def kernel(h, src, dst, graph_ids, betas, W_cls, b_cls, time_execs=0):
    global LAST_EXEC_NS
    import time as _time
    _enable_jax_compile_cache()
    from concourse.bass_utils import run_bass_kernel_spmd

    cfg = Cfg(N=40000, E=640000, G=64, NC=8)
    in_maps, counts, sched = _prep(cfg, h, src, dst, graph_ids, betas)
    nc = _get_compiled(cfg, sched)

    def _run():
        last = None
        for attempt in range(3):
            try:
                return run_bass_kernel_spmd(nc, in_maps,
                                            core_ids=list(range(cfg.NC)))
            except Exception as e:  # transient axon worker hangs
                last = e
                _time.sleep(5)
        raise last

    res = _run()
    if time_execs:
        # no NTFF profiling hook is available in this container, so report
        # median wall-clock of repeated NEFF executions (includes the axon
        # dispatch overhead; on-device time is lower)
        ts = []
        for _ in range(time_execs):
            t0 = _time.time()
            res = run_bass_kernel_spmd(nc, in_maps, core_ids=list(range(cfg.NC)))
            ts.append(_time.time() - t0)
        LAST_EXEC_NS = int(np.median(ts) * 1e9)
    pooled = np.zeros((cfg.G, 128), np.float64)
    for r in res.results:
        pooled[:, :] += r["pooled"][:cfg.G].astype(np.float64)
    hg = (pooled / np.maximum(counts, 1.0)[:, None]).astype(np.float32)
    return hg @ np.asarray(W_cls, np.float32) + np.asarray(b_cls, np.float32)
